# revision 96
# baseline (speedup 1.0000x reference)
"""Multi-head attention kernel for Trainium2, sharded over 8 NeuronCores.

Problem: B=4, S=2048, D=256, H=8 dense transformer attention block
(per-head K/V/Q Linear projections + dot-product attention + output Linear).

Sharding: core = (batch b, head-group g); core 2*b+g handles batch b and
heads [4g, 4g+4). Each core computes its heads' contribution to the final
output Linear (Wo rows h::H belong to head h); the host sums the two
partial outputs per batch and adds the (host-folded) bias.

Algebraic folds (host-side, exact up to rounding):
  - scores = k M q^T + ku[m] + (per-query terms that cancel in softmax),
    with M = 64 * Wk (Wq/16)^T and ku = k (Wk bq/16). The x64 keeps the
    fp8 t-projection in e4m3's normal range; exp() divides it back out
    via its scale operand. No Q projection, no K/Q bias adds on chip.
  - AV+output: w^T (v Wv + bv) Wo_h = w^T (v W2) + bv Wo_h with
    W2 = 16 * Wv Wo_h; bo' = bo + sum_h bv[h] Wo_h is added on host. The
    x16 is cancelled by summing the softmax denominator with a 16-valued
    ones matmul.
  - k/v/q are transposed to [D, S] on host; q is quantized to fp8 e4m3
    and k/v to bf16 on host.

On-chip per core: projections run in bf16 (full PE rate); scores, AV and
the softmax denominator run as fp8 e4m3 DoubleRow matmuls (2 k-tiles per
instruction, 0.5 cycles/column = 4x the bf16 rate). exp() runs on the
Act engine only (no table swaps; a dummy exp at t=0 hoists the table
load), one instruction per (key-tile, query half): [128, 1024] across
two PSUM banks, with the folded K-side bias ku as its per-partition bias
operand. Scores are emitted key-tile-major so one exp covers two query
blocks with a single bias column. Softmax denominators come from a
16-valued fp8 DoubleRow ones-matmul on the PE.

The Act engine is the critical path (128 exps x ~1.04us); everything
else hides under it via a slot schedule: each of a head's 32 score slots
carries at most one extra PE task (a projection group of the next head,
or one piece - denominator / AV half - of a previous query block's AV).
The drain is minimized by pre-running the last head's nb2 AV j<6
accumulation in the final slots, shipping the last head's nb2/nb3
contributions through a separate bf16 output (host adds them), and
splitting the last exp so finishers start half an exp earlier.
"""

import numpy as np
import ml_dtypes
from contextlib import ExitStack

import concourse.bacc as bacc
import concourse.bass as bass
import concourse.tile as tile
from concourse import mybir
from concourse.bass_utils import run_bass_kernel_spmd

B, S, D, H = 4, 2048, 256, 8
P = 128
DC = D // P            # 2 contraction/e-tile chunks
HPC = H // 2           # 4 heads per core
QB = 512               # query-block width
NQB = S // QB          # 4 query blocks
MT = S // P            # 16 key tiles
F32 = mybir.dt.float32
BF16 = mybir.dt.bfloat16
E4 = mybir.dt.float8e4
EXP = mybir.ActivationFunctionType.Exp
DR = mybir.MatmulPerfMode.DoubleRow


def build_program(repeat=1, nwarm=24):
    nc = bacc.Bacc(None, target_bir_lowering=False)

    ktd = nc.dram_tensor("kt", [D, S], BF16, kind="ExternalInput")
    vtd = nc.dram_tensor("vt", [D, S], BF16, kind="ExternalInput")
    qtd = nc.dram_tensor("qt", [D, S], E4, kind="ExternalInput")
    wmd = nc.dram_tensor("wm", [HPC, D, D], BF16, kind="ExternalInput")
    w2d = nc.dram_tensor("w2", [HPC, D, D], BF16, kind="ExternalInput")
    kud = nc.dram_tensor("ku", [HPC, P, MT], F32, kind="ExternalInput")
    outd = nc.dram_tensor("out", [D, S], F32, kind="ExternalOutput")
    # Last head's nb2/nb3 contribution, summed on host: keeps the final
    # out_acc adds off the exp-gated drain chain. bf16: it is one head's
    # (normalized, so small) share, and halves the drain transfers.
    outd2 = nc.dram_tensor("out2", [D, S], BF16, kind="ExternalOutput")

    with ExitStack() as ctx:
        tc = ctx.enter_context(tile.TileContext(nc))
        const = ctx.enter_context(tc.tile_pool(name="const", bufs=1))
        wpool = ctx.enter_context(tc.tile_pool(name="w", bufs=2))
        # V2 needs 3 bufs: head h's V2 is read by av_nb during head h+1's
        # slots, while the projection for head h+2 writes a third buffer.
        kqv = ctx.enter_context(tc.tile_pool(name="kqv", bufs=3))
        epool = ctx.enter_context(tc.tile_pool(name="exp", bufs=2))
        rcpool = ctx.enter_context(tc.tile_pool(name="recip", bufs=2))
        scpool = ctx.enter_context(tc.tile_pool(name="scratch", bufs=4))
        psE = ctx.enter_context(
            tc.tile_pool(name="psE", bufs=2, space=bass.MemorySpace.PSUM))
        psR = ctx.enter_context(
            tc.tile_pool(name="psR", bufs=2, space=bass.MemorySpace.PSUM))
        psM = ctx.enter_context(
            tc.tile_pool(name="psM", bufs=2, space=bass.MemorySpace.PSUM))

        ones_w = const.tile([P, P], BF16)
        nc.vector.memset(ones_w[:], 1.0)
        ones16 = const.tile([P, 2, P], E4)
        nc.vector.memset(ones16[:], 16.0)

        for _rep in range(repeat):
            _build_iteration(nc, const, wpool, kqv, epool, rcpool, scpool,
                             psE, psR, psM, ones_w, ones16,
                             ktd, vtd, qtd, wmd, w2d, kud, outd, outd2,
                             nwarm)

    nc.compile()
    return nc


def _build_iteration(nc, const, wpool, kqv, epool, rcpool, scpool,
                     psE, psR, psM, ones_w, ones16,
                     ktd, vtd, qtd, wmd, w2d, kud, outd, outd2, nwarm=24):
    # Warm the PE through the cold p-state window during the input-DMA wait.
    ps_warm = psM.tile([P, QB], F32, tag="psM")
    for wi in range(nwarm):
        nc.tensor.matmul(ps_warm[:, :P], ones_w[:], ones_w[:],
                         start=(wi == 0), stop=(wi == nwarm - 1))
    # Dummy exp with no DMA deps: pulls the compiler-inserted activation
    # table load to t~0 instead of just before the first real exp.
    act_warm = scpool.tile([P, 1], F32, tag="actwarm", bufs=1)
    nc.scalar.activation(act_warm[:], ones_w[:, 0:1], EXP)

    def dc_pair(dram, sl):
        """[p, dc, cols] view of a [D, S]-layout DRAM tensor's column
        range — both 128-row chunks in one strided DMA."""
        return dram[:, sl].rearrange("(dc p) c -> p dc c", dc=DC)

    def load_weights(h, first=False):
        wm_sb = wpool.tile([P, DC, D], BF16, tag="wm")
        w2_sb = wpool.tile([P, DC, D], BF16, tag="w2")
        ku_sb = wpool.tile([P, MT], F32, tag="ku")
        nc.sync.dma_start(wm_sb[:], wmd[h].rearrange("(dc p) e -> p dc e",
                                                     dc=DC))
        if not first:
            nc.sync.dma_start(ku_sb[:], kud[h])
            nc.sync.dma_start(w2_sb[:], w2d[h].rearrange(
                "(dc p) e -> p dc e", dc=DC))
        return wm_sb, w2_sb, ku_sb

    # Head 0: every DMA before the first exp is on the critical path.
    # Sync queue: kT quarter 0 (t-proj group 0/1), then wm, ku, the rest
    # of kT, and w2 (first needed at slot 16) last. gpsimd queue: all of
    # qT (scores), then vT (V2 groups from slot 16) — vT transfers
    # otherwise delay kT's on the shared DMA engines.
    kT = const.tile([P, DC, S], BF16)
    vT = const.tile([P, DC, S], BF16)
    qT = const.tile([P, DC, S], E4)
    HS = S // 2
    # Micro-cascade for the first score tiles: kT arrives in 256-column
    # slivers so micro t-projections can feed exp(mt0..3) while the bulk
    # transfers stream in behind them.
    w_cur = load_weights(0, first=True)
    nc.sync.dma_start(kT[:, :, 0:D], dc_pair(ktd, slice(0, D)))
    nc.sync.dma_start(kT[:, :, D:QB], dc_pair(ktd, slice(D, QB)))
    nc.sync.dma_start(kT[:, :, QB:2 * QB], dc_pair(ktd, slice(QB, 2 * QB)))
    nc.sync.dma_start(w_cur[2][:], kud[0])
    nc.sync.dma_start(kT[:, :, HS:], dc_pair(ktd, slice(HS, S)))
    nc.sync.dma_start(w_cur[1][:], w2d[0].rearrange("(dc p) e -> p dc e",
                                                    dc=DC))
    # gpsimd queue continues: first query block (first exp), then the
    # rest of q, then v.
    nc.gpsimd.dma_start(qT[:, :, 0:QB], dc_pair(qtd, slice(0, QB)))
    nc.gpsimd.dma_start(qT[:, :, QB:2 * QB], dc_pair(qtd, slice(QB, 2 * QB)))
    nc.gpsimd.dma_start(qT[:, :, HS:], dc_pair(qtd, slice(HS, S)))
    for half in range(2):
        sl = slice(half * HS, (half + 1) * HS)
        nc.gpsimd.dma_start(vT[:, :, sl], dc_pair(vtd, sl))

    # out_acc[p, et, n] accumulates out^T[f = et*128+p, n] over heads
    out_acc = const.tile([P, DC, S], F32)

    def proj_group(g, wm_sb, w2_sb, tT_h, V2_h):
        """One projection PSUM group of head h: g<8 -> t-proj (mb=g//2,
        et=g%2); g>=8 -> V2 (mp=g-8, two m-tiles per bank)."""
        ps = psM.tile([P, QB], F32, tag="psM")
        if g < 8:
            mb, et = g // 2, g % 2
            for dc in range(DC):
                nc.tensor.matmul(
                    ps[:], wm_sb[:, dc, et * P:(et + 1) * P],
                    kT[:, dc, mb * QB:(mb + 1) * QB],
                    start=(dc == 0), stop=(dc == DC - 1))
            nc.vector.tensor_copy(tT_h[:, et, mb * QB:(mb + 1) * QB], ps[:])
        else:
            mp = g - 8
            for half in range(2):
                mt = 2 * mp + half
                for dc in range(DC):
                    nc.tensor.matmul(
                        ps[:, half * D:(half + 1) * D],
                        vT[:, dc, mt * P:(mt + 1) * P],
                        w2_sb[:, dc, :],
                        start=(dc == 0), stop=(dc == DC - 1))
            nc.vector.tensor_copy(V2_h[:, 2 * mp:2 * mp + 2, :], ps[:])

    def scores_slot(nbp, mt, tT_h, ku_sb, expT, split=False):
        """Two DR score matmuls (query blocks 2*nbp, 2*nbp+1) for key tile
        mt into one 2-bank PSUM tile, then a single exp over both (two
        halves when split, so the drain chain starts one half earlier)."""
        ps = psE.tile([P, 2 * QB], F32, tag="psE")
        for half in range(2):
            nb = 2 * nbp + half
            nc.tensor.matmul(
                ps[:, half * QB:(half + 1) * QB],
                tT_h[:, :, mt * P:(mt + 1) * P],
                qT[:, :, nb * QB:(nb + 1) * QB],
                start=True, stop=True, perf_mode=DR)
        base = nbp * 2 * QB
        if split:
            for half in range(2):
                nc.scalar.activation(
                    expT[:, mt, base + half * QB:base + (half + 1) * QB],
                    ps[:, half * QB:(half + 1) * QB],
                    EXP, bias=ku_sb[:, mt:mt + 1], scale=1.0 / 64.0)
        else:
            nc.scalar.activation(
                expT[:, mt, base:base + 2 * QB], ps[:],
                EXP, bias=ku_sb[:, mt:mt + 1], scale=1.0 / 64.0)

    JH = MT // 2

    def denom_mm(nb, expT, psS, jlo, jhi):
        esl = slice(nb * QB, (nb + 1) * QB)
        for j in range(jlo, jhi):
            nc.tensor.matmul(psS[:], ones16[:],
                             expT[:, 2 * j:2 * j + 2, esl],
                             start=(j == 0), stop=(j == JH - 1),
                             perf_mode=DR)

    def et_mm(nb, et, V2_h, expT, ps, jlo, jhi):
        esl = slice(nb * QB, (nb + 1) * QB)
        for j in range(jlo, jhi):
            nc.tensor.matmul(
                ps[:], V2_h[:, 2 * j:2 * j + 2, et * P:(et + 1) * P],
                expT[:, 2 * j:2 * j + 2, esl],
                start=(j == 0), stop=(j == JH - 1), perf_mode=DR)

    def av_denom(nb, expT):
        """Softmax-denominator piece: 16-valued ones DR matmul + recip."""
        psS = psM.tile([P, QB], F32, tag="psM")
        denom_mm(nb, expT, psS, 0, JH)
        recip = rcpool.tile([P, QB], F32, tag="recip")
        nc.vector.reciprocal(recip[:], psS[:])
        return recip

    def av_et(h, nb, et, V2_h, expT, recip):
        """One AV output tile (et half) + normalize into out_acc."""
        ps = psR.tile([P, QB], F32, tag="psR")
        et_mm(nb, et, V2_h, expT, ps, 0, JH)
        av_norm(h, nb, et, ps, recip)

    def av_norm(h, nb, et, ps, recip):
        esl = slice(nb * QB, (nb + 1) * QB)
        osl = out_acc[:, et, esl]
        if h == 0:
            nc.vector.tensor_mul(osl, ps[:], recip[:])
        elif h == HPC - 1 and nb >= 2:
            # Drain path: ship this head's contribution separately (host
            # adds it) instead of serializing an on-chip add; bf16 halves
            # the exp-gated output transfer.
            scb = scpool.tile([P, QB], BF16, tag="scb")
            nc.vector.tensor_mul(scb[:], ps[:], recip[:])
            nc.sync.dma_start(outd2[et * P:(et + 1) * P,
                                    nb * QB:(nb + 1) * QB], scb[:])
            return
        else:
            sc = scpool.tile([P, QB], F32, tag="sc")
            nc.vector.tensor_mul(sc[:], ps[:], recip[:])
            nc.gpsimd.tensor_add(osl, osl, sc[:])
        if h == HPC - 1 or (h == HPC - 2 and nb >= 2):
            # nb2/nb3's out_acc holds only heads 0..HPC-2: DMA it right
            # after the HPC-2 accumulation, off the drain chain.
            eng = nc.sync if et == 0 else nc.gpsimd
            eng.dma_start(outd[et * P:(et + 1) * P, nb * QB:(nb + 1) * QB],
                          osl)

    tT_cur = kqv.tile([P, DC, S], E4, tag="tT")
    V2_cur = kqv.tile([P, MT, D], E4, tag="V2")
    exp_prev = V2_prev = None
    proj_queue = []        # deferred projection groups, one per free slot
    for h in range(HPC):
        wm_sb, w2_sb, ku_sb = w_cur
        if h + 1 < HPC:
            w_next = load_weights(h + 1)
            tT_nxt = kqv.tile([P, DC, S], E4, tag="tT")
            V2_nxt = kqv.tile([P, MT, D], E4, tag="V2")
            proj_queue.extend(
                (g, w_next[0], w_next[1], tT_nxt, V2_nxt)
                for g in range(16))

        def micro_tproj(c):
            """128x256 t-projection micro-group pair (both et) for key
            columns [c*256, (c+1)*256): head 0's first four score tiles."""
            for et in range(DC):
                ps = psM.tile([P, D], F32, tag="psM")
                for dc in range(DC):
                    nc.tensor.matmul(
                        ps[:], wm_sb[:, dc, et * P:(et + 1) * P],
                        kT[:, dc, c * D:(c + 1) * D],
                        start=(dc == 0), stop=(dc == DC - 1))
                nc.vector.tensor_copy(tT_cur[:, et, c * D:(c + 1) * D],
                                      ps[:])

        if h == 0:
            micro_tproj(0)

        expT = epool.tile([P, MT, S], E4, tag="exp")

        # Slot schedule: each of the 32 score slots carries at most ONE
        # extra PE task (<= ~0.9us) so the Act engine's single-buffer
        # psE lookahead never starves: either a projection group of the
        # next head, or one piece (denom / AV-et0 / AV-et1) of a pending
        # query block's AV.
        av_pieces = {}  # slot -> (h', nb, piece)
        if h > 0:
            for nb in range(NQB):
                base = 1 + 8 * nb
                av_pieces[base] = (h - 1, nb, 0)
                av_pieces[base + 1] = (h - 1, nb, 1)
                av_pieces[base + 2] = (h - 1, nb, 2)
        held = {}
        if h == HPC - 1:
            # Own nb0/nb1 (only need the nbp=0 exps) drain early; nb2's
            # denominator/AV accumulations over the first 12 key tiles
            # pre-run in the last slots so only the j>=6 finishers chase
            # the final exp. Slots avoid the previous head's pieces.
            for base, nb in ((20, 0), (23, 1)):
                av_pieces[base] = (h, nb, 0)
                av_pieces[base + 1] = (h, nb, 1)
                av_pieces[base + 2 if nb == 0 else base + 5] = (h, nb, 2)
            av_pieces[29] = (h, 2, 3)   # partial denom nb2
            av_pieces[30] = (h, 2, 4)   # partial AV et0 nb2
            av_pieces[31] = (h, 2, 5)   # partial AV et1 nb2
        recips = {}
        slot = 0
        for nbp in range(2):
            for mt in range(MT):
                if h == 0:
                    # Head 0's own projections interleave here: t-proj
                    # group pairs just ahead of the scores needing them,
                    # V2 groups through the second half. The first four
                    # score tiles come from 256-wide micro-groups (one
                    # emitted pre-loop, the second after scores mt0).
                    if nbp == 0 and mt == 0:
                        pass
                    elif nbp == 0 and mt % 4 == 0:
                        proj_group(2 * (mt // 4), wm_sb, w2_sb,
                                   tT_cur, V2_cur)
                        proj_group(2 * (mt // 4) + 1, wm_sb, w2_sb,
                                   tT_cur, V2_cur)
                    elif nbp == 1 and mt % 2 == 0:
                        proj_group(8 + mt // 2, wm_sb, w2_sb,
                                   tT_cur, V2_cur)
                scores_slot(nbp, mt, tT_cur, ku_sb, expT,
                            split=(h == HPC - 1 and nbp == 1
                                   and mt == MT - 1))
                if h == 0 and nbp == 0 and mt == 0:
                    micro_tproj(1)
                if slot in av_pieces:
                    ah, anb, piece = av_pieces[slot]
                    a_e, a_v = ((expT, V2_cur) if ah == h
                                else (exp_prev, V2_prev))
                    if piece == 0:
                        recips[(ah, anb)] = av_denom(anb, a_e)
                    elif piece <= 2:
                        av_et(ah, anb, piece - 1, a_v, a_e,
                              recips[(ah, anb)])
                    elif piece == 3:
                        psS = psM.tile([P, QB], F32, tag="psM")
                        denom_mm(anb, a_e, psS, 0, 6)
                        held["d", anb] = psS
                    else:
                        ps = psR.tile([P, QB], F32, tag="psR")
                        et_mm(anb, piece - 4, a_v, a_e, ps, 0, 6)
                        held["e", anb, piece - 4] = ps
                elif proj_queue and (h > 0 or slot >= 16):
                    proj_group(*proj_queue.pop(0))
                slot += 1

        exp_prev, V2_prev = expT, V2_cur
        if h + 1 < HPC:
            tT_cur, V2_cur = tT_nxt, V2_nxt
            w_cur = w_next

    # Drain: nb2's j>=6 finishers plus all of nb3, ordered so the only
    # work chasing the final exp is a handful of matmuls and the norm/DMA
    # chains (whose adds were moved to the host via out2). nb3's AV
    # accumulates into a free psE-pool tile so it doesn't wait for nb2's
    # psR banks, and its DMAs take the faster sync queue.
    psS3 = psM.tile([P, QB], F32, tag="psM")
    denom_mm(3, exp_prev, psS3, 0, 6)
    psS2 = held["d", 2]
    denom_mm(2, exp_prev, psS2, 6, JH)
    recip2 = rcpool.tile([P, QB], F32, tag="recip")
    nc.vector.reciprocal(recip2[:], psS2[:])
    for et in range(DC):
        et_mm(2, et, V2_prev, exp_prev, held["e", 2, et], 6, JH)
    denom_mm(3, exp_prev, psS3, 6, JH)
    recip3 = rcpool.tile([P, QB], F32, tag="recip")
    nc.vector.reciprocal(recip3[:], psS3[:])
    ps3 = psE.tile([P, 2 * QB], F32, tag="psE")
    for et in range(DC):
        et_mm(3, et, V2_prev, exp_prev, ps3[:, et * QB:(et + 1) * QB], 0, JH)
    for et in range(DC):
        av_norm(HPC - 1, 2, et, held["e", 2, et], recip2)
    for et in range(DC):
        scb = scpool.tile([P, QB], BF16, tag="scb")
        nc.vector.tensor_mul(scb[:], ps3[:, et * QB:(et + 1) * QB],
                             recip3[:])
        nc.sync.dma_start(outd2[et * P:(et + 1) * P, 3 * QB:4 * QB], scb[:])


_progs = {}


def _get_prog(repeat=1):
    if repeat not in _progs:
        _progs[repeat] = build_program(repeat)
    return _progs[repeat]


def _prepare_in_maps(k, v, q, Wk, bk, Wv, bv, Wq, bq, Wo, bo):
    scale = np.float32(1.0 / 16.0)  # 1/sqrt(D), exact power of two
    in_maps = []
    for core in range(2 * B):
        b, g = core // 2, core % 2
        hs = list(range(g * HPC, (g + 1) * HPC))
        wm = np.stack([
            (Wk[h].astype(np.float64)
             @ (Wq[h].astype(np.float64) * scale).T * 64.0
             ).astype(ml_dtypes.bfloat16)
            for h in hs])
        w2 = np.stack([
            (Wv[h].astype(np.float64)
             @ Wo[h::H].astype(np.float64) * 16.0).astype(ml_dtypes.bfloat16)
            for h in hs])
        ku = np.stack([
            (k[b].astype(np.float64)
             @ (Wk[h].astype(np.float64) @ (bq[h].astype(np.float64) * scale))
             ).astype(np.float32).reshape(MT, P).T
            for h in hs])
        in_maps.append({
            "kt": np.ascontiguousarray(k[b].T).astype(ml_dtypes.bfloat16),
            "vt": np.ascontiguousarray(v[b].T).astype(ml_dtypes.bfloat16),
            "qt": np.ascontiguousarray(q[b].T).astype(ml_dtypes.float8_e4m3),
            "wm": np.ascontiguousarray(wm),
            "w2": np.ascontiguousarray(w2),
            "ku": np.ascontiguousarray(ku),
        })
    return in_maps


def _bo_prime(bv, Wo, bo):
    acc = bo.astype(np.float64).copy()
    for h in range(H):
        acc += bv[h].astype(np.float64) @ Wo[h::H].astype(np.float64)
    return acc.astype(np.float32)


def _run_spmd(in_maps, repeat=1, **kwargs):
    nc = _get_prog(repeat)
    return run_bass_kernel_spmd(nc, in_maps, core_ids=list(range(2 * B)),
                                **kwargs)


def kernel(k, v, q, Wk, bk, Wv, bv, Wq, bq, Wo, bo):
    arrs = [np.asarray(x, dtype=np.float32)
            for x in (k, v, q, Wk, bk, Wv, bv, Wq, bq, Wo, bo)]
    k, v, q, Wk, bk, Wv, bv, Wq, bq, Wo, bo = arrs
    in_maps = _prepare_in_maps(k, v, q, Wk, bk, Wv, bv, Wq, bq, Wo, bo)
    rr = _run_spmd(in_maps)
    bop = _bo_prime(bv, Wo, bo)
    out = np.empty((B, S, D), np.float32)
    for b in range(B):
        r0, r1 = rr.results[2 * b], rr.results[2 * b + 1]
        out[b] = (r0["out"].T + r1["out"].T
                  + r0["out2"].astype(np.float32).T
                  + r1["out2"].astype(np.float32).T + bop)
    return out
